# revision 11
# baseline (speedup 1.0000x reference)
"""GravityField Trainium2 kernel.

out = U * sqrt(1 + clip(0.1 * grav, -0.9, 5) + 1e-6)
where grav[t] = phi[t] . sum_t'(phi[t'] * mass[t']), phi = sqrt(2/R)*cos(coords@W+b),
mass = softplus(relu(coords@w1+b1)@w2+b2).

Sharding: pure data-parallel over B (8 batches -> 8 cores, no communication).
Each core processes coords [8192, 64] and U [8192, 512] (= 64*8 flattened).

v2 structure (vs baseline): Sin is the only in-loop ACT op (one table load);
softplus is batched post-loop (second load) and Sqrt is phase 2's (third) --
the baseline's per-chunk Sigmoid/Ln/Sin cycling cost 48 x 1283 ns of table
reloads.  mass pre-act uses a fused DVE relu*w2 + GpSimd partition_all_reduce
instead of PE matmuls; the [1,512]->[128,4] scale transpose goes through one
SBUF->SBUF DMA reshape + a K=4 PE transpose instead of 4 K=1 matmuls; U moves
in [128, 2048] tiles (one DMA issue per 512 tokens each way).
"""

import sys

sys.path.insert(0, "/opt/trn_rl_repo")

import numpy as np
from contextlib import ExitStack

import concourse.bass as bass
import concourse.bacc as bacc
import concourse.mybir as mybir
from concourse import tile
from concourse import bass_isa
from concourse.bass_utils import run_bass_kernel_spmd
from concourse.masks import make_identity

F32 = mybir.dt.float32
F16 = mybir.dt.float16
AF = mybir.ActivationFunctionType
ALU = mybir.AluOpType

B, T, D, R_LR, N_RFF = 8, 8192, 64, 8, 64
F = D * R_LR  # 512 floats of U per (b, t)
STRENGTH = 0.1
HALF_PI = 1.5707963267948966
TWO_PI = 6.283185307179586
INV_2PI = 0.15915494309189535
MAGIC = 12582912.0  # 1.5 * 2**23: fp32 add/sub rounds to nearest integer
PI_CLAMP = 3.14159  # strictly inside [-pi, pi] for the ACT Sin table
BIGC = 512
N_BIG = T // BIGC  # 16
CHUNK = 128
PHI_SUM_SCALE = STRENGTH * 2.0 / N_RFF


def build_program():
    nc = bacc.Bacc("TRN2", target_bir_lowering=False, debug=False, num_devices=8)

    u_d = nc.dram_tensor("U", [T, F], F32, kind="ExternalInput")
    coords_d = nc.dram_tensor("coords", [T, D], F32, kind="ExternalInput")
    w1_d = nc.dram_tensor("mass_w1", [D, D], F32, kind="ExternalInput")
    b1_d = nc.dram_tensor("mass_b1", [D], F32, kind="ExternalInput")
    w2_d = nc.dram_tensor("mass_w2", [D, 1], F32, kind="ExternalInput")
    b2_d = nc.dram_tensor("mass_b2", [1], F32, kind="ExternalInput")
    rffw_d = nc.dram_tensor("rff_W", [D, N_RFF], F32, kind="ExternalInput")
    rffb_d = nc.dram_tensor("rff_b", [N_RFF], F32, kind="ExternalInput")
    out_d = nc.dram_tensor("out", [T, F], F32, kind="ExternalOutput")

    with tile.TileContext(nc) as tc, ExitStack() as ctx:
        const = ctx.enter_context(tc.tile_pool(name="const", bufs=1))

        identity = const.tile([128, 128], F32)
        make_identity(nc, identity[:])

        # stationary operands must have one producing engine (PE LW micro-op
        # encodes a single semaphore wait) -> bounce DMA'd weights off DVE
        w_stage = const.tile([65, 128], F32)
        nc.sync.dma_start(w_stage[0:64, 0:64], w1_d[:, :])
        nc.sync.dma_start(w_stage[64:65, 0:64], b1_d[None, :])
        nc.sync.dma_start(w_stage[0:64, 64:128], rffw_d[:, :])
        nc.sync.dma_start(w_stage[64:65, 64:128], rffb_d[None, :])
        nc.vector.tensor_scalar_add(w_stage[64:65, 64:128], w_stage[64:65, 64:128], HALF_PI)
        w_comb = const.tile([65, 128], F32)
        nc.vector.tensor_copy(w_comb[:], w_stage[:])

        w2col = const.tile([D, 1], F32)
        nc.sync.dma_start(w2col[:], w2_d[:, :])
        b2col = const.tile([D, 1], F32)
        nc.sync.dma_start(b2col[:], b2_d[None, :].to_broadcast((D, 1)))
        sqrt_bias = const.tile([128, 1], F32)
        nc.vector.memset(sqrt_bias[:], 1.000001)

        phi_sum = const.tile([N_RFF, 1], F32)
        phiT_all = const.tile([N_RFF, T], F32)   # sin args in-loop, then phi in-place
        mass_all = const.tile([N_RFF, T], F16)  # mass bcast to 64 rows
        partials = const.tile([N_RFF, N_BIG], F32)

        # manually double-buffered caug so the ones row is written only once
        caug_a = const.tile([D + 1, BIGC], F32)
        caug_b = const.tile([D + 1, BIGC], F32)
        caug2 = [caug_a, caug_b]
        for cb in caug2:
            nc.vector.memset(cb[D : D + 1, :], 1.0)

        ct_pool = ctx.enter_context(tc.tile_pool(name="ct", bufs=3))
        mw_pool = ctx.enter_context(tc.tile_pool(name="mw", bufs=2))
        mp_pool = ctx.enter_context(tc.tile_pool(name="mp", bufs=2))
        phw_pool = ctx.enter_context(tc.tile_pool(name="phw", bufs=2))
        u_pool = ctx.enter_context(tc.tile_pool(name="u", bufs=16))
        sc_pool = ctx.enter_context(tc.tile_pool(name="sc", bufs=2))

        u_tiles = []

        with (
            tc.tile_pool(name="ptr", bufs=2, space=bass.MemorySpace.PSUM) as ptr_pool,
            tc.tile_pool(name="pbig", bufs=2, space=bass.MemorySpace.PSUM) as pbig_pool,
        ):
            for c in range(N_BIG):
                tsl = slice(c * BIGC, (c + 1) * BIGC)

                ct = ct_pool.tile([128, 4 * D], F32, tag="ct")
                src = coords_d[tsl, :].rearrange("(j p) d -> p j d", p=128)
                nc.sync.dma_start(ct[:].rearrange("p (j d) -> p j d", j=4), src)

                ut = u_pool.tile([CHUNK, 4 * F], F32, tag="u")
                nc.sync.dma_start(
                    ut[:].rearrange("p (j f) -> p j f", j=4),
                    u_d[tsl, :].rearrange("(j p) f -> p j f", p=128),
                )
                u_tiles.append(ut)

                tp = ptr_pool.tile([D, BIGC], F32, tag="tp")
                for j in range(4):
                    nc.tensor.transpose(
                        tp[:, j * 128 : (j + 1) * 128],
                        ct[:, j * D : (j + 1) * D],
                        identity[:],
                    )
                caug = caug2[c % 2]
                nc.scalar.copy(caug[0:D, :], tp[:])  # ACT, table-free

                big = pbig_pool.tile([128, BIGC], F32, tag="big")
                nc.tensor.matmul(big[:], w_comb[:], caug[:], start=True, stop=True)

                # mass: relu(h)*w2 fused on DVE, GpSimd partition all-reduce
                # (result lands broadcast on 64 rows), then in-loop
                # softplus = ln(1 + exp(mpre + b2)) -- exp and ln share one
                # ACT table so the loop body never reloads tables
                mw = mw_pool.tile([D, BIGC], F32, tag="mw")
                nc.vector.tensor_scalar(
                    mw[:], big[0:D, :], 0.0, w2col[:], op0=ALU.max, op1=ALU.mult
                )
                mp = mp_pool.tile([D, BIGC], F32, tag="mp")
                nc.gpsimd.partition_all_reduce(
                    mp[:], mw[:], D, bass_isa.ReduceOp.add
                )
                nc.scalar.activation(mp[:], mp[:], AF.Exp, bias=b2col[:])
                nc.vector.tensor_scalar_add(mp[:], mp[:], 1.0)
                nc.scalar.activation(mass_all[:, tsl], mp[:], AF.Ln)

                # range-reduce x -> [-pi, pi]: y = x - 2pi*round(x/2pi)
                # chain runs in place in phiT_all, alternating DVE/Pool;
                # Sin itself happens post-loop (its table clashes with exp/ln)
                x = big[D : 2 * D, :]
                k = phiT_all[:, tsl]
                nc.vector.tensor_scalar(
                    k, x, INV_2PI, MAGIC, op0=ALU.mult, op1=ALU.add
                )
                nc.gpsimd.tensor_scalar(
                    k, k, MAGIC, -TWO_PI, op0=ALU.subtract, op1=ALU.mult
                )
                nc.vector.tensor_tensor(k, x, k, op=ALU.add)
                nc.gpsimd.tensor_scalar(
                    k, k, PI_CLAMP, -PI_CLAMP, op0=ALU.min, op1=ALU.max
                )

            # tail: batched Sin (one table load) + fused scale-mul-reduce
            for c in range(N_BIG):
                tsl = slice(c * BIGC, (c + 1) * BIGC)
                nc.scalar.activation(phiT_all[:, tsl], phiT_all[:, tsl], AF.Sin)
                phw = phw_pool.tile([D, BIGC], F32, tag="phw")
                nc.vector.scalar_tensor_tensor(
                    phw[:],
                    phiT_all[:, tsl],
                    PHI_SUM_SCALE,
                    mass_all[:, tsl],
                    op0=ALU.mult,
                    op1=ALU.mult,
                    accum_out=partials[:, c : c + 1],
                )
            nc.vector.reduce_sum(phi_sum[:], partials[:], axis=mybir.AxisListType.X)

        with (
            tc.tile_pool(name="ppg", bufs=2, space=bass.MemorySpace.PSUM) as ppg_pool,
        ):
            for g in range(N_BIG):
                tsl = slice(g * BIGC, (g + 1) * BIGC)
                # influence directly in token-major [128, 4] columns:
                # pg[:, j] = phiT[:, sub128].T @ phi_sum (scales pre-folded)
                pg = ppg_pool.tile([128, 4], F32, tag="pg")
                for j in range(4):
                    csl = slice(g * BIGC + j * 128, g * BIGC + (j + 1) * 128)
                    nc.tensor.matmul(
                        pg[:, j : j + 1],
                        phiT_all[:, csl],
                        phi_sum[:],
                        start=True,
                        stop=True,
                    )
                sc4 = sc_pool.tile([128, 4], F32, tag="sc4")
                nc.vector.tensor_scalar(
                    sc4[:], pg[:], -0.9, 5.0, op0=ALU.max, op1=ALU.min
                )
                nc.scalar.activation(sc4[:], sc4[:], AF.Sqrt, bias=sqrt_bias[:])

                ut = u_tiles[g]
                for j in range(4):
                    usl = slice(j * F, (j + 1) * F)
                    scj = sc4[:, j : j + 1]
                    if j < 2:
                        nc.vector.tensor_scalar_mul(ut[:, usl], ut[:, usl], scj)
                    elif j == 2:
                        nc.scalar.mul(ut[:, usl], ut[:, usl], scj)
                    else:
                        nc.gpsimd.tensor_scalar_mul(ut[:, usl], ut[:, usl], scj)
                nc.sync.dma_start(
                    out_d[tsl, :].rearrange("(j p) f -> p j f", p=128),
                    ut[:].rearrange("p (j f) -> p j f", j=4),
                )

    nc.compile()
    return nc


_NC_CACHE = None


def _get_program():
    global _NC_CACHE
    if _NC_CACHE is None:
        _NC_CACHE = build_program()
    return _NC_CACHE


def run(inputs: dict, trace: bool = False, tmpdir=None):
    nc = _get_program()
    U = np.ascontiguousarray(np.asarray(inputs["U"], dtype=np.float32)).reshape(B, T, F)
    coords = np.ascontiguousarray(np.asarray(inputs["coords"], dtype=np.float32))
    shared = {
        "mass_w1": np.ascontiguousarray(np.asarray(inputs["mass_w1"], np.float32)),
        "mass_b1": np.ascontiguousarray(np.asarray(inputs["mass_b1"], np.float32)),
        "mass_w2": np.ascontiguousarray(np.asarray(inputs["mass_w2"], np.float32)),
        "mass_b2": np.ascontiguousarray(np.asarray(inputs["mass_b2"], np.float32)),
        "rff_W": np.ascontiguousarray(np.asarray(inputs["rff_W"], np.float32)),
        "rff_b": np.ascontiguousarray(np.asarray(inputs["rff_b"], np.float32)),
    }
    in_maps = [{"U": U[i], "coords": coords[i], **shared} for i in range(B)]
    res = run_bass_kernel_spmd(nc, in_maps, list(range(B)), trace=trace, tmpdir=tmpdir)
    out = np.stack([res.results[i]["out"].reshape(T, D, R_LR) for i in range(B)])
    return out.astype(np.float32), res


def kernel(**inputs) -> np.ndarray:
    out, _ = run(inputs, trace=False)
    return out


# revision 13
# speedup vs baseline: 1.9159x; 1.9159x over previous
"""GravityField Trainium2 kernel.

out = U * sqrt(1 + clip(0.1 * grav, -0.9, 5) + 1e-6)
where grav[t] = phi[t] . sum_t'(phi[t'] * mass[t']), phi = sqrt(2/R)*cos(coords@W+b),
mass = softplus(relu(coords@w1+b1)@w2+b2).

Sharding: pure data-parallel over B (8 batches -> 8 cores, no communication).
Each core processes coords [8192, 64] and U [8192, 512] (= 64*8 flattened).

v4 structure (vs baseline):
- ACT tables: the loop body only uses Copy/Relu/Exp/Ln (exp+ln share one
  table, relu/copy live in every table) so there is a single in-loop table
  load; Sin is batched post-loop and Sqrt once in phase 2.  The baseline's
  per-chunk Sigmoid/Ln/Sin cycle cost 48 x 1283 ns in reloads.
- softplus(x) = ln(exp(x) + 1) with the +1 folded into Ln's bias operand.
- coords transposes are packed two-at-a-time (stationary [128,128] instead
  of [128,64]) halving PE transpose time; quadrant unpack splits ACT/DVE.
- grav is computed directly in token-major [128,4] columns via K=64 N=1
  matmuls with phiT slices stationary (replaces [1,512] matmuls + K=1
  transposes; 41us -> 22us of PE).
- U moves in [128, 2048] tiles (one DMA issue per 512 tokens each way).
- GpSimd only runs a homogeneous stream of clamp tensor_scalars: mixing op
  families on the Q7 costs ~5-7us per switch (library reload), and
  partition_all_reduce measures ~4.4us, so both are avoided entirely.
"""

import sys

sys.path.insert(0, "/opt/trn_rl_repo")

import numpy as np
from contextlib import ExitStack

import concourse.bass as bass
import concourse.bacc as bacc
import concourse.mybir as mybir
from concourse import tile
from concourse.bass_utils import run_bass_kernel_spmd
from concourse.masks import make_identity

F32 = mybir.dt.float32
F32R = mybir.dt.float32r
AF = mybir.ActivationFunctionType
ALU = mybir.AluOpType

B, T, D, R_LR, N_RFF = 8, 8192, 64, 8, 64
F = D * R_LR  # 512 floats of U per (b, t)
STRENGTH = 0.1
HALF_PI = 1.5707963267948966
TWO_PI = 6.283185307179586
INV_2PI = 0.15915494309189535
MAGIC = 12582912.0  # 1.5 * 2**23: fp32 add/sub rounds to nearest integer
PI_CLAMP = 3.14159  # strictly inside [-pi, pi] for the ACT Sin table
BIGC = 512
N_BIG = T // BIGC  # 16
CHUNK = 128
PHI_SUM_SCALE = STRENGTH * 2.0 / N_RFF


def build_program():
    nc = bacc.Bacc("TRN2", target_bir_lowering=False, debug=False, num_devices=8)

    u_d = nc.dram_tensor("U", [T, F], F32, kind="ExternalInput")
    coords_d = nc.dram_tensor("coords", [T, D], F32, kind="ExternalInput")
    w1_d = nc.dram_tensor("mass_w1", [D, D], F32, kind="ExternalInput")
    b1_d = nc.dram_tensor("mass_b1", [D], F32, kind="ExternalInput")
    w2_d = nc.dram_tensor("mass_w2", [D, 1], F32, kind="ExternalInput")
    b2_d = nc.dram_tensor("mass_b2", [1], F32, kind="ExternalInput")
    rffw_d = nc.dram_tensor("rff_W", [D, N_RFF], F32, kind="ExternalInput")
    rffb_d = nc.dram_tensor("rff_b", [N_RFF], F32, kind="ExternalInput")
    out_d = nc.dram_tensor("out", [T, F], F32, kind="ExternalOutput")
    mscr_d = nc.dram_tensor("mscr", [N_BIG, BIGC], F32)  # mass broadcast bounce

    with tile.TileContext(nc) as tc, ExitStack() as ctx:
        const = ctx.enter_context(tc.tile_pool(name="const", bufs=1))

        identity = const.tile([128, 128], F32)
        make_identity(nc, identity[:])

        # stationary operands must have one producing engine (PE LW micro-op
        # encodes a single semaphore wait) -> bounce DMA'd weights off DVE
        w_stage = const.tile([65, 128], F32)
        nc.sync.dma_start(w_stage[0:64, 0:64], w1_d[:, :])
        nc.sync.dma_start(w_stage[64:65, 0:64], b1_d[None, :])
        nc.sync.dma_start(w_stage[0:64, 64:128], rffw_d[:, :])
        nc.sync.dma_start(w_stage[64:65, 64:128], rffb_d[None, :])
        nc.vector.tensor_scalar_add(w_stage[64:65, 64:128], w_stage[64:65, 64:128], HALF_PI)
        w_comb = const.tile([65, 128], F32)
        nc.vector.tensor_copy(w_comb[:], w_stage[:])

        w2_stage = const.tile([D, 1], F32)
        nc.sync.dma_start(w2_stage[:], w2_d[:, :])
        w2col = const.tile([D, 1], F32)
        nc.vector.tensor_copy(w2col[:], w2_stage[:])

        b2_sb = const.tile([1, 1], F32)
        nc.sync.dma_start(b2_sb[:], b2_d[None, :])
        one11 = const.tile([1, 1], F32)
        nc.vector.memset(one11[:], 1.0)
        sqrt_bias = const.tile([128, 1], F32)
        nc.vector.memset(sqrt_bias[:], 1.000001)

        phi_sum = const.tile([N_RFF, 1], F32)
        phiT_all = const.tile([N_RFF, T], F32)  # sin args in-loop, then phi in-place
        partials = const.tile([N_RFF, N_BIG], F32)

        # manually double-buffered caug so the ones row is written only once
        caug_a = const.tile([D + 1, BIGC], F32)
        caug_b = const.tile([D + 1, BIGC], F32)
        caug2 = [caug_a, caug_b]
        for cb in caug2:
            nc.vector.memset(cb[D : D + 1, :], 1.0)

        ct_pool = ctx.enter_context(tc.tile_pool(name="ct", bufs=3))
        hT_pool = ctx.enter_context(tc.tile_pool(name="hT", bufs=2))
        et_pool = ctx.enter_context(tc.tile_pool(name="et", bufs=2))
        mas_pool = ctx.enter_context(tc.tile_pool(name="mas", bufs=2))
        bc_pool = ctx.enter_context(tc.tile_pool(name="bc", bufs=2))
        phw_pool = ctx.enter_context(tc.tile_pool(name="phw", bufs=2))
        u_pool = ctx.enter_context(tc.tile_pool(name="u", bufs=16))
        sc_pool = ctx.enter_context(tc.tile_pool(name="sc", bufs=2))

        u_tiles = []

        with (
            tc.tile_pool(name="ptr", bufs=2, space=bass.MemorySpace.PSUM) as ptr_pool,
            tc.tile_pool(name="pbig", bufs=2, space=bass.MemorySpace.PSUM) as pbig_pool,
            tc.tile_pool(name="pmT", bufs=2, space=bass.MemorySpace.PSUM) as pmT_pool,
        ):
            for c in range(N_BIG):
                tsl = slice(c * BIGC, (c + 1) * BIGC)

                ct = ct_pool.tile([128, 4 * D], F32, tag="ct")
                src = coords_d[tsl, :].rearrange("(j p) d -> p j d", p=128)
                nc.sync.dma_start(ct[:].rearrange("p (j d) -> p j d", j=4), src)

                ut = u_pool.tile([CHUNK, 4 * F], F32, tag="u")
                nc.sync.dma_start(
                    ut[:].rearrange("p (j f) -> p j f", j=4),
                    u_d[tsl, :].rearrange("(j p) f -> p j f", p=128),
                )
                u_tiles.append(ut)

                # two packed transposes: stationary [128, 128] covers two
                # 128-token subtiles; quadrants unpacked by ACT (same
                # partitions) and DVE (cross partition) copies
                tp = ptr_pool.tile([128, 2 * 128], F32, tag="tp")
                nc.tensor.transpose(tp[:, 0:128], ct[:, 0:128], identity[:])
                nc.tensor.transpose(tp[:, 128:256], ct[:, 128:256], identity[:])
                caug = caug2[c % 2]
                nc.scalar.copy(caug[0:D, 0:128], tp[0:D, 0:128])
                nc.vector.tensor_copy(caug[0:D, 128:256], tp[D:128, 0:128])
                nc.scalar.copy(caug[0:D, 256:384], tp[0:D, 128:256])
                nc.vector.tensor_copy(caug[0:D, 384:512], tp[D:128, 128:256])

                big = pbig_pool.tile([128, BIGC], F32, tag="big")
                nc.tensor.matmul(big[:], w_comb[:], caug[:], start=True, stop=True)

                # mass pre-act via PE (w2 stationary, fp32r single-pass),
                # then softplus = Ln(Exp(pre + b2) + 1) on ACT: exp and ln
                # share one table; +1 rides Ln's bias; +b2 rides Exp's
                hT = hT_pool.tile([D, BIGC], F32, tag="hT")
                nc.scalar.activation(hT[:], big[0:D, :], AF.Relu)
                mT = pmT_pool.tile([1, BIGC], F32, tag="mT")
                nc.tensor.matmul(mT[:], w2col[:], hT[:], start=True, stop=True)
                et = et_pool.tile([1, BIGC], F32, tag="et")
                nc.scalar.activation(et[:], mT[:], AF.Exp, bias=b2_sb[:])
                mas = mas_pool.tile([1, BIGC], F32, tag="mas")
                nc.scalar.activation(mas[:], et[:], AF.Ln, bias=one11[:])
                nc.sync.dma_start(mscr_d[c : c + 1, :], mas[:])

                # range-reduce x -> [-pi, pi]: y = x - 2pi*round(x/2pi),
                # in place in phiT_all; final clamp on GpSimd (its only op
                # family, so no Q7 library switching); Sin happens post-loop
                x = big[D : 2 * D, :]
                k = phiT_all[:, tsl]
                nc.vector.tensor_scalar(
                    k, x, INV_2PI, MAGIC, op0=ALU.mult, op1=ALU.add
                )
                nc.vector.tensor_scalar(
                    k, k, MAGIC, -TWO_PI, op0=ALU.subtract, op1=ALU.mult
                )
                nc.vector.tensor_tensor(k, x, k, op=ALU.add)
                nc.gpsimd.tensor_scalar(
                    k, k, PI_CLAMP, -PI_CLAMP, op0=ALU.min, op1=ALU.max
                )

            # tail: batched Sin (one table load) + mass broadcast re-read
            # from DRAM (0-stride partition read) + fused scale-mul-reduce
            for c in range(N_BIG):
                tsl = slice(c * BIGC, (c + 1) * BIGC)
                bc = bc_pool.tile([N_RFF, BIGC], F32, tag="bc")
                nc.sync.dma_start(bc[:], mscr_d[c : c + 1, :].to_broadcast((N_RFF, BIGC)))
                nc.scalar.activation(phiT_all[:, tsl], phiT_all[:, tsl], AF.Sin)
                phw = phw_pool.tile([D, BIGC], F32, tag="phw")
                nc.vector.scalar_tensor_tensor(
                    phw[:],
                    phiT_all[:, tsl],
                    PHI_SUM_SCALE,
                    bc[:],
                    op0=ALU.mult,
                    op1=ALU.mult,
                    accum_out=partials[:, c : c + 1],
                )
            nc.vector.reduce_sum(phi_sum[:], partials[:], axis=mybir.AxisListType.X)

        with (
            tc.tile_pool(name="ppg", bufs=2, space=bass.MemorySpace.PSUM) as ppg_pool,
        ):
            for g in range(N_BIG):
                # influence directly in token-major [128, 4] columns:
                # pg[:, j] = phiT[:, sub128].T @ phi_sum (scales pre-folded)
                pg = ppg_pool.tile([128, 4], F32, tag="pg")
                for j in range(4):
                    csl = slice(g * BIGC + j * 128, g * BIGC + (j + 1) * 128)
                    nc.tensor.matmul(
                        pg[:, j : j + 1],
                        phiT_all[:, csl],
                        phi_sum[:],
                        start=True,
                        stop=True,
                    )
                sc4 = sc_pool.tile([128, 4], F32, tag="sc4")
                nc.vector.tensor_scalar(
                    sc4[:], pg[:], -0.9, 5.0, op0=ALU.max, op1=ALU.min
                )
                nc.scalar.activation(sc4[:], sc4[:], AF.Sqrt, bias=sqrt_bias[:])

                ut = u_tiles[g]
                for j in range(4):
                    usl = slice(j * F, (j + 1) * F)
                    scj = sc4[:, j : j + 1]
                    if j < 2:
                        nc.vector.tensor_scalar_mul(ut[:, usl], ut[:, usl], scj)
                    else:
                        nc.scalar.mul(ut[:, usl], ut[:, usl], scj)
                tsl = slice(g * BIGC, (g + 1) * BIGC)
                nc.sync.dma_start(
                    out_d[tsl, :].rearrange("(j p) f -> p j f", p=128),
                    ut[:].rearrange("p (j f) -> p j f", j=4),
                )

    nc.compile()
    return nc


_NC_CACHE = None


def _get_program():
    global _NC_CACHE
    if _NC_CACHE is None:
        _NC_CACHE = build_program()
    return _NC_CACHE


def run(inputs: dict, trace: bool = False, tmpdir=None):
    nc = _get_program()
    U = np.ascontiguousarray(np.asarray(inputs["U"], dtype=np.float32)).reshape(B, T, F)
    coords = np.ascontiguousarray(np.asarray(inputs["coords"], dtype=np.float32))
    shared = {
        "mass_w1": np.ascontiguousarray(np.asarray(inputs["mass_w1"], np.float32)),
        "mass_b1": np.ascontiguousarray(np.asarray(inputs["mass_b1"], np.float32)),
        "mass_w2": np.ascontiguousarray(np.asarray(inputs["mass_w2"], np.float32)),
        "mass_b2": np.ascontiguousarray(np.asarray(inputs["mass_b2"], np.float32)),
        "rff_W": np.ascontiguousarray(np.asarray(inputs["rff_W"], np.float32)),
        "rff_b": np.ascontiguousarray(np.asarray(inputs["rff_b"], np.float32)),
    }
    in_maps = [{"U": U[i], "coords": coords[i], **shared} for i in range(B)]
    res = run_bass_kernel_spmd(nc, in_maps, list(range(B)), trace=trace, tmpdir=tmpdir)
    out = np.stack([res.results[i]["out"].reshape(T, D, R_LR) for i in range(B)])
    return out.astype(np.float32), res


def kernel(**inputs) -> np.ndarray:
    out, _ = run(inputs, trace=False)
    return out
